# revision 33
# baseline (speedup 1.0000x reference)
"""Trainium2 Bass kernel: single attention head, data-parallel over batch.

Shards the [8, 2048, 1024] input over 8 NeuronCores (1 batch element each,
projection weights replicated), runs a fused attention kernel per core, and
gathers the [8, 2048, 64] output.

Per-core math (X [S,F], W* [F,D]):
  qT/kT/vT = (X @ W + b)^T  computed as  W_chunk^T-stationary @ XT-moving.
  sT[k,q] = kT_tile^T @ qT  (contract d=64)
  e = exp(sT * 1/sqrt(S) + mask_bias[k])      one fused ACT op; masked k rows
                                              get bias -1e9 -> e == 0.
  ctxT_aug[65,q] += v_aug[k,:]^T @ e          v_aug has a ones column, so row
                                              64 accumulates the softmax denom.
  out[q,:] = transpose(ctxT_aug)[:, :64] * (1/denom)
No running max is needed: scores/sqrt(S) are O(0.3) for this problem, and
masked lanes underflow exp() to exactly 0.0, matching the reference softmax.

Modes:
  "bf16": X is pre-cast to bf16 host-side and loaded via DMA-transpose
          (X^T never touches the PE); all big matmuls run bf16 at
          1 cycle/column with fast weight loads.
  "f32r": X loaded fp32 and transposed on the PE; matmuls in float32r
          (TF32-like). ~2 cycles/column measured, higher precision.
"""

import math

import numpy as np

_B, _S, _F, _D = 8, 2048, 1024, 64
_NT = _S // 128  # 16 key/seq tiles
_FC = _F // 128  # 8 contraction chunks
_NQ = _S // 512  # 4 query chunks
_SCALE = 1.0 / math.sqrt(float(_S))
_NEG = np.float32(-1.0e9)

_MODE = "bf16"


def _ensure_path():
    try:
        import concourse.bass  # noqa: F401

        return
    except ImportError:
        pass
    import sys

    for p in ("/opt/trn_rl_repo", "/root/.axon_site/_ro/trn_rl_repo"):
        if p not in sys.path:
            sys.path.insert(0, p)
    import concourse.bass  # noqa: F401


def build_program(mode=_MODE):
    _ensure_path()
    from contextlib import ExitStack

    import concourse.bacc as bacc
    import concourse.mybir as mybir
    from concourse.masks import make_identity
    from concourse.tile import TileContext

    dt = mybir.dt
    f32 = dt.float32
    mmdt = {"bf16": dt.bfloat16, "f32r": dt.float32r, "f32": dt.float32}[mode]
    xdt = dt.bfloat16 if mode == "bf16" else f32
    AF = mybir.ActivationFunctionType

    # Bacc (not Bass): its finalize() runs the event-semaphore pass that
    # consolidates >2 per-instruction sem waits, which walrus rejects.
    nc = bacc.Bacc()
    # bf16 mode: host delivers X already transposed ([F, S]) and the weights
    # pre-cast to bf16; loads are then plain contiguous DMAs.
    xshape = [_F, _S] if mode == "bf16" else [_S, _F]
    x_d = nc.dram_tensor("x", xshape, xdt, kind="ExternalInput")
    if mode == "bf16":
        # Wq|Wk packed into one [F, 128] stationary (halves projection
        # weight-loads); bq|bk packed to a [128,1] bias.
        wqk_d = nc.dram_tensor("wqk", [_F, 2 * _D], mmdt, kind="ExternalInput")
        wv_d = nc.dram_tensor("wv", [_F, _D], mmdt, kind="ExternalInput")
        bqk_d = nc.dram_tensor("bqk", [2 * _D, 1], f32, kind="ExternalInput")
        bv_d = nc.dram_tensor("bv", [_D, 1], f32, kind="ExternalInput")
    else:
        wq_d = nc.dram_tensor("wq", [_F, _D], mmdt, kind="ExternalInput")
        wk_d = nc.dram_tensor("wk", [_F, _D], mmdt, kind="ExternalInput")
        wv_d = nc.dram_tensor("wv", [_F, _D], mmdt, kind="ExternalInput")
        bq_d = nc.dram_tensor("bq", [_D, 1], f32, kind="ExternalInput")
        bk_d = nc.dram_tensor("bk", [_D, 1], f32, kind="ExternalInput")
        bv_d = nc.dram_tensor("bv", [_D, 1], f32, kind="ExternalInput")
    mb_d = nc.dram_tensor("mb", [128, _NT], f32, kind="ExternalInput")
    out_d = nc.dram_tensor("out", [_S, _D], f32, kind="ExternalOutput")

    with ExitStack() as ctx:
        tc = ctx.enter_context(TileContext(nc))
        consts = ctx.enter_context(tc.tile_pool(name="consts", bufs=1))
        xload = ctx.enter_context(tc.tile_pool(name="xload", bufs=3))
        xtp = ctx.enter_context(tc.tile_pool(name="xtp", bufs=1))
        projp = ctx.enter_context(tc.tile_pool(name="projp", bufs=1))
        epool = ctx.enter_context(tc.tile_pool(name="epool", bufs=6))
        smalls = ctx.enter_context(tc.tile_pool(name="smalls", bufs=4))
        outp = ctx.enter_context(tc.tile_pool(name="outp", bufs=4))
        # Shared PSUM tag (4 banks) + 4 persistent ctx accumulator banks =
        # all 8 PSUM banks. bf16 mode uses 4 single-bank slots (scores tiles
        # are [128,512]); f32r mode needs 2-bank slots for the X-transpose
        # staging tiles.
        psb = ctx.enter_context(tc.tile_pool(name="psb", bufs=2, space="PSUM"))
        psc = ctx.enter_context(tc.tile_pool(name="psc", bufs=1, space="PSUM"))

        # Small loads go through SWDGE (gpsimd) so the two HWDGE queues
        # (sync, scalar) stay clear for the X transpose stream.
        ident = consts.tile([128, 128], f32)
        make_identity(nc, ident)
        ident_m = consts.tile([128, 128], mmdt)
        nc.vector.tensor_copy(ident_m, ident)
        if mode == "bf16":
            w_qk = consts.tile([128, _FC, 2 * _D], mmdt)
            nc.gpsimd.dma_start(
                out=w_qk, in_=wqk_d[:, :].rearrange("(c p) d -> p c d", p=128)
            )
            w_v = consts.tile([128, _FC, _D], mmdt)
            nc.gpsimd.dma_start(
                out=w_v, in_=wv_d[:, :].rearrange("(c p) d -> p c d", p=128)
            )
            b_qk = consts.tile([2 * _D, 1], f32)
            nc.gpsimd.dma_start(out=b_qk, in_=bqk_d[:, :])
            b_v = consts.tile([_D, 1], f32)
            nc.gpsimd.dma_start(out=b_v, in_=bv_d[:, :])
        else:
            w_all = consts.tile([128, 3, _FC, _D], mmdt)
            nc.sync.dma_start(
                out=w_all[:, 0], in_=wq_d[:, :].rearrange("(c p) d -> p c d", p=128)
            )
            nc.sync.dma_start(
                out=w_all[:, 1], in_=wk_d[:, :].rearrange("(c p) d -> p c d", p=128)
            )
            nc.sync.dma_start(
                out=w_all[:, 2], in_=wv_d[:, :].rearrange("(c p) d -> p c d", p=128)
            )
            w_q, w_k, w_v = w_all[:, 0], w_all[:, 1], w_all[:, 2]
            b_q = consts.tile([_D, 1], f32)
            nc.gpsimd.dma_start(out=b_q, in_=bq_d[:, :])
            b_k = consts.tile([_D, 1], f32)
            nc.gpsimd.dma_start(out=b_k, in_=bk_d[:, :])
            b_v = consts.tile([_D, 1], f32)
            nc.gpsimd.dma_start(out=b_v, in_=bv_d[:, :])
        mb = consts.tile([128, _NT], f32)
        nc.gpsimd.dma_start(out=mb, in_=mb_d[:, :])
        # (biases/mask stay on SWDGE: tiny, transpose-free targets)

        if mode == "bf16":
        # qk_sb rows 0-63 = qT, rows 64-127 = kT; the kT half is re-homed to
        # partition base 0 via SBUF->SBUF DMA for the scores stationary.
            qk_sb = projp.tile([128, _S], mmdt)
            qT = qk_sb[0:_D, :]
            kT = projp.tile([_D, _S], mmdt)
        else:
            qT = projp.tile([_D, _S], mmdt)
            kT = projp.tile([_D, _S], mmdt)
        vT = projp.tile([_D, _S], mmdt)
        v_sb = projp.tile([128, _NT, _D + 1], mmdt)
        ones_f = consts.tile([128, 1], f32)
        nc.vector.memset(ones_f, 1.0)
        ones_r = consts.tile([128, 1], mmdt)
        nc.vector.tensor_copy(ones_r, ones_f)
        for t in range(_NT):
            nc.vector.tensor_copy(v_sb[:, t, _D : _D + 1], ones_r)

        # Phases A+B interleaved per query-chunk: transpose-load 512 rows of
        # X, then the three projections for that 512-column chunk.
        for qc in range(_NQ):
            xt_q = xtp.tile([128, _FC, 512], mmdt, name=f"xt{qc}", tag=f"xt{qc}")
            if mode == "bf16":
                # Two half-loads on alternating HWDGE queues: keeps both DMA
                # queues streaming and lets the first projection matmuls start
                # after only half a chunk has landed (sub-tile deps).
                half = _FC // 2
                for ci, eng in ((0, nc.sync), (half, nc.scalar)):
                    eng.dma_start(
                        out=xt_q[:, ci : ci + half, :],
                        in_=x_d[
                            ci * 128 : (ci + half) * 128,
                            qc * 512 : (qc + 1) * 512,
                        ].rearrange("(c p) s -> p c s", p=128),
                    )
            else:
                for j in range(4):
                    i = qc * 4 + j
                    x_t = xload.tile([128, _F], f32, name="x_t", tag="x_t")
                    nc.sync.dma_start(out=x_t, in_=x_d[i * 128 : (i + 1) * 128, :])
                    tp = psb.tile([128, _FC, 128], f32, name="tp", tag="big")
                    for c in range(_FC):
                        nc.tensor.transpose(
                            tp[:, c, :], x_t[:, c * 128 : (c + 1) * 128], ident
                        )
                    nc.vector.tensor_copy(xt_q[:, :, j * 128 : (j + 1) * 128], tp)
            if mode == "bf16":
                pq = psb.tile([128, 512], f32, name="pq", tag="big")
                for c in range(_FC):
                    nc.tensor.matmul(
                        pq,
                        lhsT=w_qk[:, c, :],
                        rhs=xt_q[:, c, :],
                        start=(c == 0),
                        stop=(c == _FC - 1),
                    )
                nc.scalar.activation(
                    qk_sb[:, qc * 512 : (qc + 1) * 512], pq, AF.Identity, bias=b_qk
                )
                # SWDGE queue: a HWDGE-queued shift would head-of-line block
                # the later xt loads behind its evac dependency.
                nc.gpsimd.dma_start(
                    out=kT[:, qc * 512 : (qc + 1) * 512],
                    in_=qk_sb[_D : 2 * _D, qc * 512 : (qc + 1) * 512],
                )
                pv = psb.tile([_D, 512], f32, name="pv", tag="big")
                for c in range(_FC):
                    nc.tensor.matmul(
                        pv,
                        lhsT=w_v[:, c, :],
                        rhs=xt_q[:, c, :],
                        start=(c == 0),
                        stop=(c == _FC - 1),
                    )
                nc.scalar.activation(
                    vT[:, qc * 512 : (qc + 1) * 512], pv, AF.Identity, bias=b_v
                )
            else:
                for w_sb, b_sb, dstT in (
                    (w_q, b_q, qT),
                    (w_k, b_k, kT),
                    (w_v, b_v, vT),
                ):
                    pp = psb.tile([_D, 512], f32, name="pp", tag="big")
                    for c in range(_FC):
                        nc.tensor.matmul(
                            pp,
                            lhsT=w_sb[:, c, :],
                            rhs=xt_q[:, c, :],
                            start=(c == 0),
                            stop=(c == _FC - 1),
                        )
                    nc.scalar.activation(
                        dstT[:, qc * 512 : (qc + 1) * 512],
                        pp,
                        AF.Identity,
                        bias=b_sb,
                    )

        # Phase C: v back to natural [k, d] layout, plus the ones column.
        for t in range(_NT):
            tv = psb.tile([128, _D], mmdt, name="tv", tag="big")
            nc.tensor.transpose(
                tv, vT[:, t * 128 : (t + 1) * 128], ident_m[0:_D, 0:_D]
            )
            nc.vector.tensor_copy(v_sb[:, t, 0:_D], tv)

        # Phase D: flash loop over key tiles.
        cps = [
            psc.tile([_D + 1, 512], f32, name=f"cps{qc}", tag=f"cps{qc}")
            for qc in range(_NQ)
        ]
        if mode == "bf16":
            # Two 2-bank score tiles per key tile; [128,1024] exps halve the
            # per-op ACT fixed overhead (ACT is co-bound with PE here).
            for t in range(_NT):
                scs = []
                for h in range(2):
                    sc = psb.tile([128, 1024], f32, name="sc", tag="big")
                    for u in range(2):
                        qc = h * 2 + u
                        nc.tensor.matmul(
                            sc[:, u * 512 : (u + 1) * 512],
                            lhsT=kT[:, t * 128 : (t + 1) * 128],
                            rhs=qT[:, qc * 512 : (qc + 1) * 512],
                            start=True,
                            stop=True,
                        )
                    scs.append(sc)
                ets = []
                for h in range(2):
                    e_t = epool.tile([128, 1024], mmdt, name="e_t", tag="e_t")
                    nc.scalar.activation(
                        e_t, scs[h], AF.Exp, bias=mb[:, t : t + 1], scale=_SCALE
                    )
                    ets.append(e_t)
                for qc in range(_NQ):
                    nc.tensor.matmul(
                        cps[qc],
                        lhsT=v_sb[:, t, :],
                        rhs=ets[qc // 2][:, (qc % 2) * 512 : (qc % 2 + 1) * 512],
                        start=(t == 0),
                        stop=(t == _NT - 1),
                        skip_group_check=True,
                    )
        else:
            for t in range(_NT):
                for h in range(2):
                    sc = psb.tile([128, 1024], f32, name="sc", tag="big")
                    for u in range(2):
                        qc = h * 2 + u
                        nc.tensor.matmul(
                            sc[:, u * 512 : (u + 1) * 512],
                            lhsT=kT[:, t * 128 : (t + 1) * 128],
                            rhs=qT[:, qc * 512 : (qc + 1) * 512],
                            start=True,
                            stop=True,
                        )
                    e_t = epool.tile([128, 1024], mmdt, name="e_t", tag="e_t")
                    nc.scalar.activation(
                        e_t, sc, AF.Exp, bias=mb[:, t : t + 1], scale=_SCALE
                    )
                    for u in range(2):
                        qc = h * 2 + u
                        nc.tensor.matmul(
                            cps[qc],
                            lhsT=v_sb[:, t, :],
                            rhs=e_t[:, u * 512 : (u + 1) * 512],
                            start=(t == 0),
                            stop=(t == _NT - 1),
                            skip_group_check=True,
                        )

        # Phase E: transpose ctxT_aug back to [q, d], normalize, store.
        for qc in range(_NQ):
            ctxT = smalls.tile([_D + 1, 512], f32, name="ctxT", tag="ctxT")
            nc.scalar.activation(ctxT, cps[qc], AF.Copy)
            # Replace the denominator row by its reciprocal once; after the
            # transpose it lands as the per-partition scale for the multiply.
            nc.vector.reciprocal(ctxT[_D : _D + 1, :], ctxT[_D : _D + 1, :])
            for j in range(4):
                fin = psb.tile([128, _D + 1], f32, name="fin", tag="big")
                nc.tensor.transpose(
                    fin,
                    ctxT[:, j * 128 : (j + 1) * 128],
                    ident[0 : _D + 1, 0 : _D + 1],
                )
                o_t = outp.tile([128, _D], f32, name="o_t", tag="o_t")
                nc.vector.tensor_scalar_mul(o_t, fin[:, 0:_D], fin[:, _D : _D + 1])
                r0 = (qc * 4 + j) * 128
                nc.sync.dma_start(out=out_d[r0 : r0 + 128, :], in_=o_t)
    if not nc.is_finalized():
        nc.finalize()
    return nc


def prep_in_maps(inputs, mode=_MODE):
    x_full = np.asarray(inputs["input_tensor"], dtype=np.float32)
    wq = np.ascontiguousarray(np.asarray(inputs["Wq"], dtype=np.float32))
    wk = np.ascontiguousarray(np.asarray(inputs["Wk"], dtype=np.float32))
    wv = np.ascontiguousarray(np.asarray(inputs["Wv"], dtype=np.float32))
    if mode == "bf16":
        import ml_dtypes

        bf = ml_dtypes.bfloat16
        x_full = x_full.astype(bf).transpose(0, 2, 1)  # [B, F, S]
        wqk = np.ascontiguousarray(np.concatenate([wq, wk], axis=1).astype(bf))
        wv = wv.astype(bf)
    mask = np.asarray(inputs["attention_mask"])
    bq = np.ascontiguousarray(np.asarray(inputs["bq"], dtype=np.float32).reshape(_D, 1))
    bk = np.ascontiguousarray(np.asarray(inputs["bk"], dtype=np.float32).reshape(_D, 1))
    bv = np.ascontiguousarray(np.asarray(inputs["bv"], dtype=np.float32).reshape(_D, 1))
    in_maps = []
    for b in range(_B):
        mbias = np.where(mask[b, 0], _NEG, np.float32(0.0)).astype(np.float32)
        mbias = np.ascontiguousarray(mbias.reshape(_NT, 128).T)
        m = {"x": np.ascontiguousarray(x_full[b]), "mb": mbias}
        if mode == "bf16":
            m.update(
                {
                    "wqk": wqk,
                    "wv": wv,
                    "bqk": np.concatenate([bq, bk], axis=0),
                    "bv": bv,
                }
            )
        else:
            m.update(
                {"wq": wq, "wk": wk, "wv": wv, "bq": bq, "bk": bk, "bv": bv}
            )
        in_maps.append(m)
    return in_maps


def run(inputs, trace=False, mode=_MODE):
    _ensure_path()
    from concourse import bass_utils

    nc = build_program(mode=mode)
    in_maps = prep_in_maps(inputs, mode=mode)
    res = bass_utils.run_bass_kernel_spmd(nc, in_maps, list(range(_B)), trace=trace)
    out = np.stack([r["out"] for r in res.results], axis=0).astype(np.float32)
    return out, res


def kernel(**inputs):
    out, _ = run(inputs, trace=False)
    return out


# revision 34
# speedup vs baseline: 1.0704x; 1.0704x over previous
"""Trainium2 Bass kernel: single attention head, data-parallel over batch.

Shards the [8, 2048, 1024] input over 8 NeuronCores (1 batch element each,
projection weights replicated), runs a fused attention kernel per core, and
gathers the [8, 2048, 64] output.

Per-core math (X [S,F], W* [F,D]):
  qT/kT/vT = (X @ W + b)^T  computed as  W_chunk^T-stationary @ XT-moving.
  sT[k,q] = kT_tile^T @ qT  (contract d=64)
  e = exp(sT * 1/sqrt(S) + mask_bias[k])      one fused ACT op; masked k rows
                                              get bias -1e9 -> e == 0.
  ctxT_aug[65,q] += v_aug[k,:]^T @ e          v_aug has a ones column, so row
                                              64 accumulates the softmax denom.
  out[q,:] = transpose(ctxT_aug)[:, :64] * (1/denom)
No running max is needed: scores/sqrt(S) are O(0.3) for this problem, and
masked lanes underflow exp() to exactly 0.0, matching the reference softmax.

Modes:
  "bf16": X is pre-cast to bf16 host-side and loaded via DMA-transpose
          (X^T never touches the PE); all big matmuls run bf16 at
          1 cycle/column with fast weight loads.
  "f32r": X loaded fp32 and transposed on the PE; matmuls in float32r
          (TF32-like). ~2 cycles/column measured, higher precision.
"""

import math

import numpy as np

_B, _S, _F, _D = 8, 2048, 1024, 64
_NT = _S // 128  # 16 key/seq tiles
_FC = _F // 128  # 8 contraction chunks
_NQ = _S // 512  # 4 query chunks
_SCALE = 1.0 / math.sqrt(float(_S))
_NEG = np.float32(-1.0e9)

_MODE = "bf16"


def _ensure_path():
    try:
        import concourse.bass  # noqa: F401

        return
    except ImportError:
        pass
    import sys

    for p in ("/opt/trn_rl_repo", "/root/.axon_site/_ro/trn_rl_repo"):
        if p not in sys.path:
            sys.path.insert(0, p)
    import concourse.bass  # noqa: F401


def build_program(mode=_MODE):
    _ensure_path()
    from contextlib import ExitStack

    import concourse.bacc as bacc
    import concourse.mybir as mybir
    from concourse.masks import make_identity
    from concourse.tile import TileContext

    dt = mybir.dt
    f32 = dt.float32
    mmdt = {"bf16": dt.bfloat16, "f32r": dt.float32r, "f32": dt.float32}[mode]
    xdt = dt.bfloat16 if mode == "bf16" else f32
    AF = mybir.ActivationFunctionType

    # Bacc (not Bass): its finalize() runs the event-semaphore pass that
    # consolidates >2 per-instruction sem waits, which walrus rejects.
    nc = bacc.Bacc()
    # bf16 mode: host delivers X already transposed ([F, S]) and the weights
    # pre-cast to bf16; loads are then plain contiguous DMAs.
    xshape = [_F, _S] if mode == "bf16" else [_S, _F]
    x_d = nc.dram_tensor("x", xshape, xdt, kind="ExternalInput")
    if mode == "bf16":
        # Wq|Wk packed into one [F, 128] stationary (halves projection
        # weight-loads); bq|bk packed to a [128,1] bias.
        wqk_d = nc.dram_tensor("wqk", [_F, 2 * _D], mmdt, kind="ExternalInput")
        wv_d = nc.dram_tensor("wv", [_F, _D], mmdt, kind="ExternalInput")
        bqk_d = nc.dram_tensor("bqk", [2 * _D, 1], f32, kind="ExternalInput")
        bv_d = nc.dram_tensor("bv", [_D, 1], f32, kind="ExternalInput")
    else:
        wq_d = nc.dram_tensor("wq", [_F, _D], mmdt, kind="ExternalInput")
        wk_d = nc.dram_tensor("wk", [_F, _D], mmdt, kind="ExternalInput")
        wv_d = nc.dram_tensor("wv", [_F, _D], mmdt, kind="ExternalInput")
        bq_d = nc.dram_tensor("bq", [_D, 1], f32, kind="ExternalInput")
        bk_d = nc.dram_tensor("bk", [_D, 1], f32, kind="ExternalInput")
        bv_d = nc.dram_tensor("bv", [_D, 1], f32, kind="ExternalInput")
    mb_d = nc.dram_tensor("mb", [128, _NT], f32, kind="ExternalInput")
    out_d = nc.dram_tensor("out", [_S, _D], f32, kind="ExternalOutput")

    with ExitStack() as ctx:
        tc = ctx.enter_context(TileContext(nc))
        consts = ctx.enter_context(tc.tile_pool(name="consts", bufs=1))
        xload = ctx.enter_context(tc.tile_pool(name="xload", bufs=3))
        xtp = ctx.enter_context(tc.tile_pool(name="xtp", bufs=1))
        projp = ctx.enter_context(tc.tile_pool(name="projp", bufs=1))
        epool = ctx.enter_context(tc.tile_pool(name="epool", bufs=6))
        smalls = ctx.enter_context(tc.tile_pool(name="smalls", bufs=4))
        outp = ctx.enter_context(tc.tile_pool(name="outp", bufs=4))
        # Shared PSUM tag (4 banks) + 4 persistent ctx accumulator banks =
        # all 8 PSUM banks. bf16 mode uses 4 single-bank slots (scores tiles
        # are [128,512]); f32r mode needs 2-bank slots for the X-transpose
        # staging tiles.
        psb_bufs = 4 if mode == "bf16" else 2
        psb = ctx.enter_context(tc.tile_pool(name="psb", bufs=psb_bufs, space="PSUM"))
        psc = ctx.enter_context(tc.tile_pool(name="psc", bufs=1, space="PSUM"))

        # Small loads go through SWDGE (gpsimd) so the two HWDGE queues
        # (sync, scalar) stay clear for the X transpose stream.
        ident = consts.tile([128, 128], f32)
        make_identity(nc, ident)
        ident_m = consts.tile([128, 128], mmdt)
        nc.vector.tensor_copy(ident_m, ident)
        if mode == "bf16":
            w_qk = consts.tile([128, _FC, 2 * _D], mmdt)
            nc.gpsimd.dma_start(
                out=w_qk, in_=wqk_d[:, :].rearrange("(c p) d -> p c d", p=128)
            )
            w_v = consts.tile([128, _FC, _D], mmdt)
            nc.gpsimd.dma_start(
                out=w_v, in_=wv_d[:, :].rearrange("(c p) d -> p c d", p=128)
            )
            b_qk = consts.tile([2 * _D, 1], f32)
            nc.gpsimd.dma_start(out=b_qk, in_=bqk_d[:, :])
            b_v = consts.tile([_D, 1], f32)
            nc.gpsimd.dma_start(out=b_v, in_=bv_d[:, :])
        else:
            w_all = consts.tile([128, 3, _FC, _D], mmdt)
            nc.sync.dma_start(
                out=w_all[:, 0], in_=wq_d[:, :].rearrange("(c p) d -> p c d", p=128)
            )
            nc.sync.dma_start(
                out=w_all[:, 1], in_=wk_d[:, :].rearrange("(c p) d -> p c d", p=128)
            )
            nc.sync.dma_start(
                out=w_all[:, 2], in_=wv_d[:, :].rearrange("(c p) d -> p c d", p=128)
            )
            w_q, w_k, w_v = w_all[:, 0], w_all[:, 1], w_all[:, 2]
            b_q = consts.tile([_D, 1], f32)
            nc.gpsimd.dma_start(out=b_q, in_=bq_d[:, :])
            b_k = consts.tile([_D, 1], f32)
            nc.gpsimd.dma_start(out=b_k, in_=bk_d[:, :])
            b_v = consts.tile([_D, 1], f32)
            nc.gpsimd.dma_start(out=b_v, in_=bv_d[:, :])
        mb = consts.tile([128, _NT], f32)
        nc.gpsimd.dma_start(out=mb, in_=mb_d[:, :])
        # (biases/mask stay on SWDGE: tiny, transpose-free targets)

        if mode == "bf16":
        # qk_sb rows 0-63 = qT, rows 64-127 = kT; the kT half is re-homed to
        # partition base 0 via SBUF->SBUF DMA for the scores stationary.
            qk_sb = projp.tile([128, _S], mmdt)
            qT = qk_sb[0:_D, :]
            kT = projp.tile([_D, _S], mmdt)
        else:
            qT = projp.tile([_D, _S], mmdt)
            kT = projp.tile([_D, _S], mmdt)
        vT = projp.tile([_D, _S], mmdt)
        v_sb = projp.tile([128, _NT, _D + 1], mmdt)
        ones_f = consts.tile([128, 1], f32)
        nc.vector.memset(ones_f, 1.0)
        ones_r = consts.tile([128, 1], mmdt)
        nc.vector.tensor_copy(ones_r, ones_f)
        for t in range(_NT):
            nc.vector.tensor_copy(v_sb[:, t, _D : _D + 1], ones_r)

        # Phases A+B interleaved per query-chunk: transpose-load 512 rows of
        # X, then the three projections for that 512-column chunk.
        for qc in range(_NQ):
            xt_q = xtp.tile([128, _FC, 512], mmdt, name=f"xt{qc}", tag=f"xt{qc}")
            if mode == "bf16":
                # Two half-loads on alternating HWDGE queues: keeps both DMA
                # queues streaming and lets the first projection matmuls start
                # after only half a chunk has landed (sub-tile deps).
                half = _FC // 2
                for ci, eng in ((0, nc.sync), (half, nc.scalar)):
                    eng.dma_start(
                        out=xt_q[:, ci : ci + half, :],
                        in_=x_d[
                            ci * 128 : (ci + half) * 128,
                            qc * 512 : (qc + 1) * 512,
                        ].rearrange("(c p) s -> p c s", p=128),
                    )
            else:
                for j in range(4):
                    i = qc * 4 + j
                    x_t = xload.tile([128, _F], f32, name="x_t", tag="x_t")
                    nc.sync.dma_start(out=x_t, in_=x_d[i * 128 : (i + 1) * 128, :])
                    tp = psb.tile([128, _FC, 128], f32, name="tp", tag="big")
                    for c in range(_FC):
                        nc.tensor.transpose(
                            tp[:, c, :], x_t[:, c * 128 : (c + 1) * 128], ident
                        )
                    nc.vector.tensor_copy(xt_q[:, :, j * 128 : (j + 1) * 128], tp)
            if mode == "bf16":
                pq = psb.tile([128, 512], f32, name="pq", tag="big")
                for c in range(_FC):
                    nc.tensor.matmul(
                        pq,
                        lhsT=w_qk[:, c, :],
                        rhs=xt_q[:, c, :],
                        start=(c == 0),
                        stop=(c == _FC - 1),
                    )
                nc.scalar.activation(
                    qk_sb[:, qc * 512 : (qc + 1) * 512], pq, AF.Identity, bias=b_qk
                )
                # SWDGE queue: a HWDGE-queued shift would head-of-line block
                # the later xt loads behind its evac dependency.
                nc.gpsimd.dma_start(
                    out=kT[:, qc * 512 : (qc + 1) * 512],
                    in_=qk_sb[_D : 2 * _D, qc * 512 : (qc + 1) * 512],
                )
                pv = psb.tile([_D, 512], f32, name="pv", tag="big")
                for c in range(_FC):
                    nc.tensor.matmul(
                        pv,
                        lhsT=w_v[:, c, :],
                        rhs=xt_q[:, c, :],
                        start=(c == 0),
                        stop=(c == _FC - 1),
                    )
                nc.scalar.activation(
                    vT[:, qc * 512 : (qc + 1) * 512], pv, AF.Identity, bias=b_v
                )
            else:
                for w_sb, b_sb, dstT in (
                    (w_q, b_q, qT),
                    (w_k, b_k, kT),
                    (w_v, b_v, vT),
                ):
                    pp = psb.tile([_D, 512], f32, name="pp", tag="big")
                    for c in range(_FC):
                        nc.tensor.matmul(
                            pp,
                            lhsT=w_sb[:, c, :],
                            rhs=xt_q[:, c, :],
                            start=(c == 0),
                            stop=(c == _FC - 1),
                        )
                    nc.scalar.activation(
                        dstT[:, qc * 512 : (qc + 1) * 512],
                        pp,
                        AF.Identity,
                        bias=b_sb,
                    )

        # Phase C: v back to natural [k, d] layout, plus the ones column.
        for t in range(_NT):
            tv = psb.tile([128, _D], mmdt, name="tv", tag="big")
            nc.tensor.transpose(
                tv, vT[:, t * 128 : (t + 1) * 128], ident_m[0:_D, 0:_D]
            )
            nc.vector.tensor_copy(v_sb[:, t, 0:_D], tv)

        # Phase D: flash loop over key tiles.
        cps = [
            psc.tile([_D + 1, 512], f32, name=f"cps{qc}", tag=f"cps{qc}")
            for qc in range(_NQ)
        ]
        if mode == "bf16":
            # 4 single-bank score tiles in flight; one exp + one ctx matmul
            # per query chunk keeps PE dense while ACT drains the exps.
            for t in range(_NT):
                scs = []
                for qc in range(_NQ):
                    sc = psb.tile([128, 512], f32, name="sc", tag="big")
                    nc.tensor.matmul(
                        sc,
                        lhsT=kT[:, t * 128 : (t + 1) * 128],
                        rhs=qT[:, qc * 512 : (qc + 1) * 512],
                        start=True,
                        stop=True,
                    )
                    scs.append(sc)
                ets = []
                for qc in range(_NQ):
                    e_t = epool.tile([128, 512], mmdt, name="e_t", tag="e_t")
                    nc.scalar.activation(
                        e_t, scs[qc], AF.Exp, bias=mb[:, t : t + 1], scale=_SCALE
                    )
                    ets.append(e_t)
                for qc in range(_NQ):
                    nc.tensor.matmul(
                        cps[qc],
                        lhsT=v_sb[:, t, :],
                        rhs=ets[qc],
                        start=(t == 0),
                        stop=(t == _NT - 1),
                        skip_group_check=True,
                    )
        else:
            for t in range(_NT):
                for h in range(2):
                    sc = psb.tile([128, 1024], f32, name="sc", tag="big")
                    for u in range(2):
                        qc = h * 2 + u
                        nc.tensor.matmul(
                            sc[:, u * 512 : (u + 1) * 512],
                            lhsT=kT[:, t * 128 : (t + 1) * 128],
                            rhs=qT[:, qc * 512 : (qc + 1) * 512],
                            start=True,
                            stop=True,
                        )
                    e_t = epool.tile([128, 1024], mmdt, name="e_t", tag="e_t")
                    nc.scalar.activation(
                        e_t, sc, AF.Exp, bias=mb[:, t : t + 1], scale=_SCALE
                    )
                    for u in range(2):
                        qc = h * 2 + u
                        nc.tensor.matmul(
                            cps[qc],
                            lhsT=v_sb[:, t, :],
                            rhs=e_t[:, u * 512 : (u + 1) * 512],
                            start=(t == 0),
                            stop=(t == _NT - 1),
                            skip_group_check=True,
                        )

        # Phase E: transpose ctxT_aug back to [q, d], normalize, store.
        for qc in range(_NQ):
            ctxT = smalls.tile([_D + 1, 512], f32, name="ctxT", tag="ctxT")
            nc.scalar.activation(ctxT, cps[qc], AF.Copy)
            for j in range(4):
                fin = psb.tile([128, _D + 1], f32, name="fin", tag="big")
                nc.tensor.transpose(
                    fin,
                    ctxT[:, j * 128 : (j + 1) * 128],
                    ident[0 : _D + 1, 0 : _D + 1],
                )
                rcp = smalls.tile([128, 1], f32, name="rcp", tag="rcp")
                nc.vector.reciprocal(rcp, fin[:, _D : _D + 1])
                o_t = outp.tile([128, _D], f32, name="o_t", tag="o_t")
                nc.vector.tensor_scalar_mul(o_t, fin[:, 0:_D], rcp)
                r0 = (qc * 4 + j) * 128
                nc.sync.dma_start(out=out_d[r0 : r0 + 128, :], in_=o_t)
    if not nc.is_finalized():
        nc.finalize()
    return nc


def prep_in_maps(inputs, mode=_MODE):
    x_full = np.asarray(inputs["input_tensor"], dtype=np.float32)
    wq = np.ascontiguousarray(np.asarray(inputs["Wq"], dtype=np.float32))
    wk = np.ascontiguousarray(np.asarray(inputs["Wk"], dtype=np.float32))
    wv = np.ascontiguousarray(np.asarray(inputs["Wv"], dtype=np.float32))
    if mode == "bf16":
        import ml_dtypes

        bf = ml_dtypes.bfloat16
        x_full = x_full.astype(bf).transpose(0, 2, 1)  # [B, F, S]
        wqk = np.ascontiguousarray(np.concatenate([wq, wk], axis=1).astype(bf))
        wv = wv.astype(bf)
    mask = np.asarray(inputs["attention_mask"])
    bq = np.ascontiguousarray(np.asarray(inputs["bq"], dtype=np.float32).reshape(_D, 1))
    bk = np.ascontiguousarray(np.asarray(inputs["bk"], dtype=np.float32).reshape(_D, 1))
    bv = np.ascontiguousarray(np.asarray(inputs["bv"], dtype=np.float32).reshape(_D, 1))
    in_maps = []
    for b in range(_B):
        mbias = np.where(mask[b, 0], _NEG, np.float32(0.0)).astype(np.float32)
        mbias = np.ascontiguousarray(mbias.reshape(_NT, 128).T)
        m = {"x": np.ascontiguousarray(x_full[b]), "mb": mbias}
        if mode == "bf16":
            m.update(
                {
                    "wqk": wqk,
                    "wv": wv,
                    "bqk": np.concatenate([bq, bk], axis=0),
                    "bv": bv,
                }
            )
        else:
            m.update(
                {"wq": wq, "wk": wk, "wv": wv, "bq": bq, "bk": bk, "bv": bv}
            )
        in_maps.append(m)
    return in_maps


def run(inputs, trace=False, mode=_MODE):
    _ensure_path()
    from concourse import bass_utils

    nc = build_program(mode=mode)
    in_maps = prep_in_maps(inputs, mode=mode)
    res = bass_utils.run_bass_kernel_spmd(nc, in_maps, list(range(_B)), trace=trace)
    out = np.stack([r["out"] for r in res.results], axis=0).astype(np.float32)
    return out, res


def kernel(**inputs):
    out, _ = run(inputs, trace=False)
    return out
